# revision 34
# baseline (speedup 1.0000x reference)
# Trainium2 Bass kernel for per-sample channel-attention module (CAM).
#
# Reference math per sample (C=512, N=H*W=4096):
#   X = x.reshape(C, N)
#   phi = Wp X ; theta = Wt X ; g = Wg X
#   attn = softmax_rows(phi @ theta^T)          # [C, C]
#   y = attn @ g                                 # [C, N]
#   Z = (y^T).flatten().reshape(C, N)            # torch permute+view reinterpretation
#   out = gamma * (Wm @ Z) + x
#
# Algebraic restructuring (cuts PE work ~1.8x vs the naive 6-GEMM chain):
#   G = X X^T                  (Gram, [C, C])
#   L = Wp G Wt^T              (attention logits via two small GEMMs)
#   A' = softmax(L) @ Wg       (fold g-projection into attention)
#   y = A' X                   (single big GEMM)
# The torch permute+view reinterpretation is free: y^T blocks are produced
# with a stride-8 column selection of X as the stationary matmul operand, so
# each PSUM tile lands exactly on a contiguous block of Z's SBUF layout.
#
# Mixed precision (validated against the fp64 reference; the softmax here is
# a hard argmax with large top1-top2 logit gaps, so post-softmax stages are
# linear in quantization error while the logit path needs >=10 bits):
#   - logit path (X^T stream, Gram, G, T1, Wp, Wt): fp16 — fp16 weights get
#     the automatic fast-weight-load path so LDWEIGHTS hides behind matmuls
#   - attn: fp16 (fp32 PSUM + exact max-subtraction in the softmax)
#   - A', X, Z, gamma*Wm^T: fp8 e4m3 with power-of-2 scales; ZS and mask
#     GEMMs run in DoubleRow perf mode (K=256 per pass, ~2x fp16 rate)
#   - residual: fp16 x, fused into the mask PSUM drain as one DVE
#     scalar_tensor_tensor op (out = PSUM/2^16 + x -> fp16); the last q uses
#     an ACT prefill + ACT scale-copy instead to keep the tail off the DVE
# Gram exploits symmetry: only upper-triangle blocks are computed (1280 of
# 2048 moving columns per tile); the 6 lower blocks come from PE transposes.

import os
import numpy as np
import ml_dtypes

import concourse.bass as bass
import concourse.mybir as mybir
import concourse.tile as tile
from concourse import bacc
from concourse.bass_utils import run_bass_kernel_spmd
from concourse.tile import TileContext

P = 128          # partitions
C = 512          # channels
N = 4096         # spatial (64*64)
CC = C // P      # 4 channel chunks
NT = N // P      # 32 spatial tiles
QF = N // C      # 8 fold factor for the permute+view reinterpretation
FP32 = mybir.dt.float32
FP16 = mybir.dt.float16
FP8 = mybir.dt.float8e4

S_X = 8.0        # x fp8 scale
S_A = 512.0      # A' fp8 scale
S_M = 8192.0     # gamma*Wm^T fp8 scale
S_MX = S_M * S_X           # 65536: residual prefill scale
ZCAST = S_X / (S_A * S_X)  # PSUM (y*S_A*S_X) -> Z*S_X


def _build_nc():
    # All DRAM tensors are HOST-PACKED into the exact SBUF layout
    # ([128 partitions, flat free dim]) so every DMA descriptor moves a
    # 2-16KB contiguous run — 1KB fp16 rows were descriptor-rate-bound.
    nc = bacc.Bacc("TRN2", target_bir_lowering=False, debug=False, num_devices=8)
    id_d = nc.dram_tensor("id16", [P, P], FP16, kind="ExternalInput").ap()
    xt_d = nc.dram_tensor("xt16p", [P, NT * C], FP16, kind="ExternalInput").ap()
    x16_d = nc.dram_tensor("x16p", [P, CC * N], FP16, kind="ExternalInput").ap()
    x8_d = nc.dram_tensor("x8p", [P, CC * N], FP8, kind="ExternalInput").ap()
    wphiT_d = nc.dram_tensor("w_phi_tp", [P, CC * C], FP16, kind="ExternalInput").ap()
    wthetaT_d = nc.dram_tensor("w_theta_tp", [P, CC * C], FP16, kind="ExternalInput").ap()
    wg_d = nc.dram_tensor("w_gp", [P, CC * C], FP16, kind="ExternalInput").ap()
    wmT8_d = nc.dram_tensor("w_mask_tp8", [P, CC * C], FP8, kind="ExternalInput").ap()
    # out rows: (q, oc2, s, c) per partition; host unpacks to [C, N]
    out_d = nc.dram_tensor("outp", [P, QF * 2 * 2 * C], FP16, kind="ExternalOutput").ap()

    with TileContext(nc) as tc:
        _body(tc, id_d, xt_d, x16_d, x8_d, wphiT_d, wthetaT_d, wg_d, wmT8_d, out_d)
    nc.compile()
    return nc


def _body(tc, id_d, xt_d, x16_d, x8_d, wphiT_d, wthetaT_d, wg_d, wmT8_d, out_d):
    nc = tc.nc
    from contextlib import ExitStack

    with ExitStack() as ctx:
        const = ctx.enter_context(tc.tile_pool(name="const", bufs=1))
        xtp = ctx.enter_context(tc.tile_pool(name="xtp", bufs=1))
        xin = ctx.enter_context(tc.tile_pool(name="xin", bufs=1))
        wpool = ctx.enter_context(tc.tile_pool(name="wpool", bufs=1))
        mid = ctx.enter_context(tc.tile_pool(name="mid", bufs=1))
        vecs = ctx.enter_context(tc.tile_pool(name="vecs", bufs=8))
        outp = ctx.enter_context(tc.tile_pool(name="outp", bufs=4))
        psA = ctx.enter_context(tc.tile_pool(name="psA", bufs=2, space="PSUM"))
        psB = ctx.enter_context(tc.tile_pool(name="psB", bufs=2, space="PSUM"))

        # Warm source: DVE memset (no gpsimd in the startup path). ~18
        # throwaway matmuls ramp the PE p-state while DMA streams in.
        wsrc = const.tile([P, C], FP16)
        nc.vector.memset(wsrc, 1.0)
        warm = psB.tile([P, 2, C], FP32, tag="psB")
        for _ in range(11):
            nc.tensor.matmul(
                warm[:, 0, :], wsrc[:, :P], wsrc, start=True, stop=True
            )

        # ---- input DMA schedule.
        # sync HWDGE:   xt even chunks, x16 left half; output stores later.
        # scalar HWDGE: identity, xt odd chunks, w_theta, w_phi, wg, x16 right.
        # gpsimd SWDGE: x8, wmT8 (needed latest; software queue).
        id16 = const.tile([P, P], FP16)
        xt_sb = xtp.tile([P, NT, C], FP16)
        x16_sb = xin.tile([P, CC, N], FP16)
        x8_sb = xin.tile([P, CC, N], FP8)
        wphiT = wpool.tile([P, CC, C], FP16)
        wthetaT = wpool.tile([P, CC, C], FP16)
        wg16 = wpool.tile([P, CC, C], FP16)
        wmT8 = wpool.tile([P, CC, C], FP8)

        # xt in 8 four-tile chunks (4KB/partition each) alternating the two
        # HWDGE queues; weights/identity/x8 on the SWDGE path.
        NCHUNK = 8
        TPC = NT // NCHUNK
        CW = TPC * C
        for k in range(NCHUNK):
            eng = nc.sync if (k % 2 == 0) else nc.scalar
            eng.dma_start(
                out=xt_sb[:, k * TPC:(k + 1) * TPC, :],
                in_=xt_d[:, k * CW:(k + 1) * CW],
            )
        nc.gpsimd.dma_start(out=id16, in_=id_d)
        nc.gpsimd.dma_start(out=wthetaT, in_=wthetaT_d)
        nc.gpsimd.dma_start(out=wphiT, in_=wphiT_d)
        nc.gpsimd.dma_start(out=wg16, in_=wg_d)
        nc.gpsimd.dma_start(out=wmT8, in_=wmT8_d)
        nc.gpsimd.dma_start(out=x8_sb, in_=x8_d)
        HN = N // 2
        xv = x16_d.rearrange("p (cc n) -> p cc n", cc=CC)
        nc.sync.dma_start(out=x16_sb[:, :, :HN], in_=xv[:, :, :HN])
        nc.scalar.dma_start(out=x16_sb[:, :, HN:], in_=xv[:, :, HN:])

        # ---- Gram, upper triangle only: G[mc-block, 128*mc:] accumulated
        # over the 32 streamed xt tiles. gaccA = rows 0,1; gaccB = rows 2,3.
        gaccA = psA.tile([P, 2, C], FP32, tag="psA")
        gaccB = psA.tile([P, 2, C], FP32, tag="psA")
        gacc = [
            (gaccA[:, 0, :], 0), (gaccA[:, 1, :C - P], P),
            (gaccB[:, 0, :C - 2 * P], 2 * P), (gaccB[:, 1, :C - 3 * P], 3 * P),
        ]
        for t in range(NT):
            for mc in range(CC):
                dst, col0 = gacc[mc]
                nc.tensor.matmul(
                    dst,
                    xt_sb[:, t, mc * P:(mc + 1) * P],
                    xt_sb[:, t, col0:],
                    start=(t == 0),
                    stop=(t == NT - 1),
                )

        # full G (fp16) in SBUF: triangle rows + 6 transposed lower blocks
        g_sb = mid.tile([P, CC, C], FP16)
        for mc in range(CC):
            src, col0 = gacc[mc]
            if mc % 2 == 0:
                nc.scalar.activation(
                    out=g_sb[:, mc, col0:], in_=src,
                    func=mybir.ActivationFunctionType.Copy, scale=1.0,
                )
            else:
                nc.vector.tensor_copy(g_sb[:, mc, col0:], src)

        t1_sb = mid.tile([P, CC, C], FP16)
        tps = {}
        tA = psB.tile([P, 2, C], FP32, tag="psB")
        tps[3], tps[2] = tA[:, 0, :], tA[:, 1, :]
        tB = psB.tile([P, 2, C], FP32, tag="psB")
        tps[1], tps[0] = tB[:, 0, :], tB[:, 1, :]

        def t1_pass(mc):
            # T1 = G @ Wt^T (G blocks stationary). mc=3 uses only triangle
            # rows; other mc need the transposed lower blocks.
            tp = tps[mc]
            for jc in range(CC):
                nc.tensor.matmul(
                    tp,
                    g_sb[:, jc, mc * P:(mc + 1) * P],
                    wthetaT[:, jc, :],
                    start=(jc == 0),
                    stop=(jc == CC - 1),
                )
            if mc == 0:
                # last copy is on L's critical path: split across ACT + DVE
                nc.scalar.activation(
                    out=t1_sb[:, mc, :C // 2], in_=tp[:, :C // 2],
                    func=mybir.ActivationFunctionType.Copy, scale=1.0,
                )
                nc.vector.tensor_copy(t1_sb[:, mc, C // 2:], tp[:, C // 2:])
            else:
                nc.any.tensor_copy(t1_sb[:, mc, :], tp)

        t1_pass(3)
        # lower blocks (a > b): G[a, b-block] = T(G[b, a-block]); packed into
        # one fp16 PSUM tile, copied back right after each transpose so each
        # T1 pass unblocks as early as possible (T1(mc) needs blocks (*,mc)).
        gt = psA.tile([P, 2, 4, P], FP16, tag="psA")
        lower = [(3, 2), (2, 1), (3, 1), (1, 0), (2, 0), (3, 0)]

        def g_fill(i):
            a, b = lower[i]
            nc.tensor.transpose(
                gt[:, i // 4, i % 4, :],
                g_sb[:, b, a * P:(a + 1) * P],
                id16,
            )
            nc.any.tensor_copy(
                g_sb[:, a, b * P:(b + 1) * P], gt[:, i // 4, i % 4, :]
            )

        g_fill(0)
        t1_pass(2)
        g_fill(1)
        g_fill(2)
        t1_pass(1)
        g_fill(3)
        g_fill(4)
        g_fill(5)
        t1_pass(0)

        # ---- L = Wp @ T1 ; softmax rows -> attn (fp16); attn^T transposes
        # interleaved per-mc so the PE never waits on a softmax.
        attn_sb = mid.tile([P, CC, C], FP16)
        attnT16 = mid.tile([P, CC, C], FP16)
        ptA = psA.tile([P, 2, C], FP16, tag="psA")  # attnT rows dc=0,1
        ptB = psA.tile([P, 2, C], FP16, tag="psA")  # attnT rows dc=2,3
        pt = {0: ptA[:, 0, :], 1: ptA[:, 1, :], 2: ptB[:, 0, :], 3: ptB[:, 1, :]}
        lps = {}

        neg_max = {}
        sums = {}
        rinv = {}

        def l_pass(mc):
            lp = lps[mc]
            # descending ic: the first matmuls use the T1 rows copied
            # earliest (T1 runs mc=3..0), so L never waits on the last copy
            for ic in reversed(range(CC)):
                nc.tensor.matmul(
                    lp,
                    wphiT[:, ic, mc * P:(mc + 1) * P],
                    t1_sb[:, ic, :],
                    start=(ic == CC - 1),
                    stop=(ic == 0),
                )
            # only the row-max on DVE here: all four maxes queue back-to-back
            # (no reciprocal head-of-line blocking in the DVE FIFO)
            neg_max[mc] = vecs.tile([P, 1], FP32, name=f"negmax{mc}")
            nc.vector.tensor_reduce(
                out=neg_max[mc], in_=lp, axis=mybir.AxisListType.X,
                op=mybir.AluOpType.max, negate=True,
            )

        def sm_exp(mc):
            sums[mc] = vecs.tile([P, 1], FP32, name=f"sums{mc}")
            nc.scalar.activation(
                out=attn_sb[:, mc, :], in_=lps[mc],
                func=mybir.ActivationFunctionType.Exp,
                bias=neg_max[mc], scale=1.0, accum_out=sums[mc],
            )

        def sm_recip(mc):
            rinv[mc] = vecs.tile([P, 1], FP32, name=f"rinv{mc}")
            nc.vector.reciprocal(rinv[mc], sums[mc])

        def sm_mul(mc):
            nc.vector.tensor_scalar_mul(
                attn_sb[:, mc, :], attn_sb[:, mc, :], rinv[mc]
            )

        def at_pass(mc):
            for dc in range(CC):
                nc.tensor.transpose(
                    pt[dc][:, mc * P:(mc + 1) * P],
                    attn_sb[:, mc, dc * P:(dc + 1) * P],
                    id16,
                )

        # L order 3,0,1,2: softmax(3) (the attnT straggler otherwise) hides
        # under L(0)/L(1); each at_pass only needs its own softmax done.
        lpA = psB.tile([P, 2, C], FP32, tag="psB")
        lpB = psB.tile([P, 2, C], FP32, tag="psB")
        lps[3], lps[0] = lpA[:, 0, :], lpA[:, 1, :]
        lps[1], lps[2] = lpB[:, 0, :], lpB[:, 1, :]
        # Emission tuned for engine FIFO order: DVE sees max3,max0,max1,
        # recip3,mul3,max2,recip0,mul0,... so every op is ready when it
        # reaches the queue head; aT(3) slots into PE order before l(2).
        l_pass(3)
        sm_exp(3)
        l_pass(0)
        sm_exp(0)
        l_pass(1)
        sm_exp(1)
        sm_recip(3)
        sm_mul(3)
        at_pass(3)
        l_pass(2)
        sm_exp(2)
        sm_recip(0)
        sm_mul(0)
        at_pass(0)
        sm_recip(1)
        sm_mul(1)
        at_pass(1)
        sm_recip(2)
        sm_mul(2)
        at_pass(2)
        for dc in range(CC):
            nc.any.tensor_copy(attnT16[:, dc, :], pt[dc])
        # p-state filler: keep the PE clocked through the attnT-copy bubble
        # so the A'-fold matmuls run at full rate. Reuses lps[1]'s PSUM slice
        # (its softmax reads finished long ago) to avoid any WAR stall.
        for _ in range(6):
            nc.tensor.matmul(
                lps[1], wsrc[:, :P], wsrc, start=True, stop=True
            )

        # ---- A'^T[j, c] = sum_d Wg[d, j] attn[c, d]; cast to fp8 * S_A
        apT8 = mid.tile([P, CC, C], FP8)
        for half in range(2):
            ap_ps = psB.tile([P, 2, C], FP32, tag="psB")
            for sub in range(2):
                jc = half * 2 + sub
                for dc in range(CC):
                    nc.tensor.matmul(
                        ap_ps[:, sub, :],
                        wg16[:, dc, jc * P:(jc + 1) * P],
                        attnT16[:, dc, :],
                        start=(dc == 0),
                        stop=(dc == CC - 1),
                    )
                nc.scalar.activation(
                    out=apT8[:, jc, :], in_=ap_ps[:, sub, :],
                    func=mybir.ActivationFunctionType.Copy, scale=S_A,
                )

        # ---- ZS (y^T blocks in Z layout) + mask GEMM + residual + store,
        # both in fp8 DoubleRow (K=256 per pass). Software-pipelined:
        # mask(q-1) runs on PE while ZS(q)'s PSUM->fp8 casts run on ACT/DVE.
        zs8 = mid.tile([P, CC, N], FP8)
        xr8 = x8_sb.rearrange("p cc (ci m q) -> p cc ci q m", ci=CC, q=QF)

        def zs_half(q, ci2):
            # 4 DoubleRow matmuls -> one ACT cast to fp8 (Z * S_X)
            pool = psA if ci2 == 0 else psB
            zp = pool.tile([P, 2, C], FP32, tag="psA" if ci2 == 0 else "psB")
            for s in range(2):
                ci = ci2 * 2 + s
                for j2 in range(2):
                    nc.tensor.matmul(
                        zp[:, s, :],
                        xr8[:, 2 * j2:2 * j2 + 2, ci, q, :],
                        apT8[:, 2 * j2:2 * j2 + 2, :],
                        start=(j2 == 0),
                        stop=(j2 == 1),
                        perf_mode=mybir.MatmulPerfMode.DoubleRow,
                    )
            nc.scalar.activation(
                out=zs8[:, 2 * ci2:2 * ci2 + 2, q * C:(q + 1) * C],
                in_=zp, func=mybir.ActivationFunctionType.Copy,
                scale=ZCAST,
            )

        def mask_half(q, oc2, prefill=False):
            # 4 DoubleRow matmuls (PSUM = mask * S_MX), then one fused DVE
            # op: out = PSUM/S_MX + x (residual), straight to fp16.
            # The last q instead prefills the residual into PSUM (ACT) and
            # finishes with an ACT scale-copy, keeping the tail off the
            # DVE FIFO (its MULTIPLY,ADDs would serialize the epilogue).
            pool = psA if oc2 == 0 else psB
            mp = pool.tile([P, 2, C], FP32, tag="psA" if oc2 == 0 else "psB")
            if prefill:
                nc.scalar.activation(
                    out=mp,
                    in_=x16_sb[:, 2 * oc2:2 * oc2 + 2, q * C:(q + 1) * C],
                    func=mybir.ActivationFunctionType.Copy, scale=S_MX,
                )
            for s in range(2):
                oc = oc2 * 2 + s
                for i2 in range(2):
                    nc.tensor.matmul(
                        mp[:, s, :],
                        wmT8[:, 2 * i2:2 * i2 + 2, oc * P:(oc + 1) * P],
                        zs8[:, 2 * i2:2 * i2 + 2, q * C:(q + 1) * C],
                        start=(i2 == 0 and not prefill),
                        stop=(i2 == 1),
                        perf_mode=mybir.MatmulPerfMode.DoubleRow,
                        skip_group_check=prefill,
                    )
            ot = outp.tile([P, 2, C], FP16)
            if prefill:
                nc.scalar.activation(
                    out=ot, in_=mp,
                    func=mybir.ActivationFunctionType.Copy, scale=1.0 / S_MX,
                )
            else:
                nc.vector.scalar_tensor_tensor(
                    out=ot, in0=mp, scalar=1.0 / S_MX,
                    in1=x16_sb[:, 2 * oc2:2 * oc2 + 2, q * C:(q + 1) * C],
                    op0=mybir.AluOpType.mult, op1=mybir.AluOpType.add,
                )
            blk = (q * 2 + oc2) * 2 * C
            nc.sync.dma_start(out=out_d[:, blk:blk + 2 * C], in_=ot)

        for q in range(QF):
            zs_half(q, 0)
            if q > 0:
                mask_half(q - 1, 0)
            zs_half(q, 1)
            if q > 0:
                mask_half(q - 1, 1)
        mask_half(QF - 1, 0, prefill=True)
        mask_half(QF - 1, 1, prefill=True)


_NC_CACHE = {}
LAST_RESULT = None


def get_nc():
    if "nc" not in _NC_CACHE:
        _NC_CACHE["nc"] = _build_nc()
    return _NC_CACHE["nc"]


def _e4m3(a):
    return np.asarray(
        np.clip(np.asarray(a, np.float32), -448.0, 448.0),
        ml_dtypes.float8_e4m3fn,
    )


_ID16 = np.eye(P, dtype=np.float16)


def _pack(arr2d):
    """[cc*128, W] -> [128, cc*W]: per-partition-contiguous SBUF layout."""
    a = np.asarray(arr2d)
    cc = a.shape[0] // P
    return np.ascontiguousarray(
        a.reshape(cc, P, a.shape[1]).transpose(1, 0, 2).reshape(P, -1)
    )


def make_in_map(xb, w_phi_tp, w_theta_tp, w_gp, w_mask_tp8):
    """Per-core input dict; xb is one sample [C, H, W]."""
    xf = np.ascontiguousarray(xb.reshape(C, N), dtype=np.float32)
    return {
        "id16": _ID16,
        "xt16p": _pack(np.ascontiguousarray(xf.T).astype(np.float16)),
        "x16p": _pack(xf.astype(np.float16)),
        "x8p": _pack(_e4m3(xf * S_X)),
        "w_phi_tp": w_phi_tp,
        "w_theta_tp": w_theta_tp,
        "w_gp": w_gp,
        "w_mask_tp8": w_mask_tp8,
    }


def prep_weights(w_phi, w_theta, w_g, w_mask, gamma):
    w_phi_tp = _pack(np.asarray(w_phi, dtype=np.float32).T.astype(np.float16))
    w_theta_tp = _pack(np.asarray(w_theta, dtype=np.float32).T.astype(np.float16))
    w_gp = _pack(np.asarray(w_g, dtype=np.float32).astype(np.float16))
    gamma64 = float(np.asarray(gamma, dtype=np.float32).reshape(-1)[0])
    w_mask_tp8 = _pack(_e4m3(
        (np.asarray(w_mask, dtype=np.float64).T * gamma64 * S_M).astype(np.float32)
    ))
    return w_phi_tp, w_theta_tp, w_gp, w_mask_tp8


def kernel(x, w_phi, w_theta, w_g, w_mask, gamma):
    global LAST_RESULT
    x = np.ascontiguousarray(np.asarray(x, dtype=np.float32))
    B, c, h, w = x.shape
    assert (c, h * w) == (C, N), (x.shape,)

    w_phi_tp, w_theta_tp, w_gp, w_mask_tp8 = prep_weights(
        w_phi, w_theta, w_g, w_mask, gamma
    )
    nc = get_nc()
    in_maps = [
        make_in_map(x[b], w_phi_tp, w_theta_tp, w_gp, w_mask_tp8)
        for b in range(B)
    ]
    trace = bool(int(os.environ.get("KERNEL_TRACE", "0")))
    res = run_bass_kernel_spmd(nc, in_maps, list(range(B)), trace=trace)
    LAST_RESULT = res
    out = np.empty((B, c, h * w), dtype=np.float32)
    for b in range(B):
        a = np.asarray(res.results[b]["outp"], dtype=np.float32)
        # [p, (q, oc2, s, c)] -> channel = (2*oc2+s)*128+p, col = q*512+c
        out[b] = a.reshape(P, QF, 2, 2, C).transpose(2, 3, 0, 1, 4).reshape(c, h * w)
    return out.reshape(B, c, h, w)


# revision 39
# speedup vs baseline: 1.0085x; 1.0085x over previous
# Trainium2 Bass kernel for per-sample channel-attention module (CAM).
#
# Reference math per sample (C=512, N=H*W=4096):
#   X = x.reshape(C, N)
#   phi = Wp X ; theta = Wt X ; g = Wg X
#   attn = softmax_rows(phi @ theta^T)          # [C, C]
#   y = attn @ g                                 # [C, N]
#   Z = (y^T).flatten().reshape(C, N)            # torch permute+view reinterpretation
#   out = gamma * (Wm @ Z) + x
#
# Algebraic restructuring (cuts PE work ~1.8x vs the naive 6-GEMM chain):
#   G = X X^T                  (Gram, [C, C])
#   L = Wp G Wt^T              (attention logits via two small GEMMs)
#   A' = softmax(L) @ Wg       (fold g-projection into attention)
#   y = A' X                   (single big GEMM)
# The torch permute+view reinterpretation is free: y^T blocks are produced
# with a stride-8 column selection of X as the stationary matmul operand, so
# each PSUM tile lands exactly on a contiguous block of Z's SBUF layout.
#
# Mixed precision (validated against the fp64 reference; the softmax here is
# a hard argmax with large top1-top2 logit gaps, so post-softmax stages are
# linear in quantization error while the logit path needs >=10 bits):
#   - logit path (X^T stream, Gram, G, T1, Wp, Wt): fp16 — fp16 weights get
#     the automatic fast-weight-load path so LDWEIGHTS hides behind matmuls
#   - attn: fp16 (fp32 PSUM + exact max-subtraction in the softmax)
#   - A', X, Z, gamma*Wm^T: fp8 e4m3 with power-of-2 scales; ZS and mask
#     GEMMs run in DoubleRow perf mode (K=256 per pass, ~2x fp16 rate)
#   - residual: fp16 x, fused into the mask PSUM drain as one DVE
#     scalar_tensor_tensor op (out = PSUM/2^16 + x -> fp16); the last q uses
#     an ACT prefill + ACT scale-copy instead to keep the tail off the DVE
# Gram exploits symmetry: only upper-triangle blocks are computed (1280 of
# 2048 moving columns per tile); the 6 lower blocks come from PE transposes.

import os
import numpy as np
import ml_dtypes

import concourse.bass as bass
import concourse.mybir as mybir
import concourse.tile as tile
from concourse import bacc
from concourse.bass_utils import run_bass_kernel_spmd
from concourse.tile import TileContext

P = 128          # partitions
C = 512          # channels
N = 4096         # spatial (64*64)
CC = C // P      # 4 channel chunks
NT = N // P      # 32 spatial tiles
QF = N // C      # 8 fold factor for the permute+view reinterpretation
FP32 = mybir.dt.float32
FP16 = mybir.dt.float16
FP8 = mybir.dt.float8e4

S_X = 8.0        # x fp8 scale
S_A = 512.0      # A' fp8 scale
S_M = 8192.0     # gamma*Wm^T fp8 scale
S_MX = S_M * S_X           # 65536: residual prefill scale
ZCAST = S_X / (S_A * S_X)  # PSUM (y*S_A*S_X) -> Z*S_X


def _build_nc():
    # All DRAM tensors are HOST-PACKED into the exact SBUF layout
    # ([128 partitions, flat free dim]) so every DMA descriptor moves a
    # 2-16KB contiguous run — 1KB fp16 rows were descriptor-rate-bound.
    nc = bacc.Bacc("TRN2", target_bir_lowering=False, debug=False, num_devices=8)
    id_d = nc.dram_tensor("id16", [P, P], FP16, kind="ExternalInput").ap()
    xt_d = nc.dram_tensor("xt16p", [P, NT * C], FP16, kind="ExternalInput").ap()
    x16_d = nc.dram_tensor("x16p", [P, CC * N], FP16, kind="ExternalInput").ap()
    x8_d = nc.dram_tensor("x8p", [P, CC * N], FP8, kind="ExternalInput").ap()
    wphiT_d = nc.dram_tensor("w_phi_tp", [P, CC * C], FP16, kind="ExternalInput").ap()
    wthetaT_d = nc.dram_tensor("w_theta_tp", [P, CC * C], FP16, kind="ExternalInput").ap()
    wg_d = nc.dram_tensor("w_gp", [P, CC * C], FP16, kind="ExternalInput").ap()
    wmT8_d = nc.dram_tensor("w_mask_tp8", [P, CC * C], FP8, kind="ExternalInput").ap()
    # out rows: (q, oc2, s, c) per partition; host unpacks to [C, N]
    out_d = nc.dram_tensor("outp", [P, QF * 2 * 2 * C], FP16, kind="ExternalOutput").ap()

    with TileContext(nc) as tc:
        _body(tc, id_d, xt_d, x16_d, x8_d, wphiT_d, wthetaT_d, wg_d, wmT8_d, out_d)
    nc.compile()
    return nc


def _body(tc, id_d, xt_d, x16_d, x8_d, wphiT_d, wthetaT_d, wg_d, wmT8_d, out_d):
    nc = tc.nc
    from contextlib import ExitStack

    with ExitStack() as ctx:
        const = ctx.enter_context(tc.tile_pool(name="const", bufs=1))
        xtp = ctx.enter_context(tc.tile_pool(name="xtp", bufs=1))
        xin = ctx.enter_context(tc.tile_pool(name="xin", bufs=1))
        wpool = ctx.enter_context(tc.tile_pool(name="wpool", bufs=1))
        mid = ctx.enter_context(tc.tile_pool(name="mid", bufs=1))
        vecs = ctx.enter_context(tc.tile_pool(name="vecs", bufs=8))
        outp = ctx.enter_context(tc.tile_pool(name="outp", bufs=4))
        psA = ctx.enter_context(tc.tile_pool(name="psA", bufs=2, space="PSUM"))
        psB = ctx.enter_context(tc.tile_pool(name="psB", bufs=2, space="PSUM"))

        # Warm source: DVE memset (no gpsimd in the startup path). ~18
        # throwaway matmuls ramp the PE p-state while DMA streams in.
        wsrc = const.tile([P, C], FP16)
        nc.vector.memset(wsrc, 1.0)
        warm = psB.tile([P, 2, C], FP32, tag="psB")
        for _ in range(11):
            nc.tensor.matmul(
                warm[:, 0, :], wsrc[:, :P], wsrc, start=True, stop=True
            )

        # ---- input DMA schedule.
        # sync HWDGE:   xt even chunks, x16 left half; output stores later.
        # scalar HWDGE: identity, xt odd chunks, w_theta, w_phi, wg, x16 right.
        # gpsimd SWDGE: x8, wmT8 (needed latest; software queue).
        id16 = const.tile([P, P], FP16)
        xt_sb = xtp.tile([P, NT, C], FP16)
        x16_sb = xin.tile([P, CC, N], FP16)
        x8_sb = xin.tile([P, CC, N], FP8)
        wphiT = wpool.tile([P, CC, C], FP16)
        wthetaT = wpool.tile([P, CC, C], FP16)
        wg16 = wpool.tile([P, CC, C], FP16)
        wmT8 = wpool.tile([P, CC, C], FP8)

        # xt in 8 four-tile chunks (4KB/partition each) alternating the two
        # HWDGE queues; weights/identity/x8 on the SWDGE path.
        NCHUNK = 8
        TPC = NT // NCHUNK
        CW = TPC * C
        for k in range(NCHUNK):
            eng = nc.sync if (k % 2 == 0) else nc.scalar
            eng.dma_start(
                out=xt_sb[:, k * TPC:(k + 1) * TPC, :],
                in_=xt_d[:, k * CW:(k + 1) * CW],
            )
        nc.gpsimd.dma_start(out=id16, in_=id_d)
        nc.gpsimd.dma_start(out=wthetaT, in_=wthetaT_d)
        nc.gpsimd.dma_start(out=wphiT, in_=wphiT_d)
        nc.gpsimd.dma_start(out=wg16, in_=wg_d)
        nc.gpsimd.dma_start(out=wmT8, in_=wmT8_d)
        nc.gpsimd.dma_start(out=x8_sb, in_=x8_d)
        HN = N // 2
        xv = x16_d.rearrange("p (cc n) -> p cc n", cc=CC)
        nc.sync.dma_start(out=x16_sb[:, :, :HN], in_=xv[:, :, :HN])
        nc.scalar.dma_start(out=x16_sb[:, :, HN:], in_=xv[:, :, HN:])

        # ---- Gram, upper triangle only: G[mc-block, 128*mc:] accumulated
        # over the 32 streamed xt tiles. gaccA = rows 0,1; gaccB = rows 2,3.
        gaccA = psA.tile([P, 2, C], FP32, tag="psA")
        gaccB = psA.tile([P, 2, C], FP32, tag="psA")
        gacc = [
            (gaccA[:, 0, :], 0), (gaccA[:, 1, :C - P], P),
            (gaccB[:, 0, :C - 2 * P], 2 * P), (gaccB[:, 1, :C - 3 * P], 3 * P),
        ]
        for t in range(NT):
            for mc in range(CC):
                dst, col0 = gacc[mc]
                nc.tensor.matmul(
                    dst,
                    xt_sb[:, t, mc * P:(mc + 1) * P],
                    xt_sb[:, t, col0:],
                    start=(t == 0),
                    stop=(t == NT - 1),
                )

        # full G (fp16) in SBUF: triangle rows + 6 transposed lower blocks
        g_sb = mid.tile([P, CC, C], FP16)
        for mc in range(CC):
            src, col0 = gacc[mc]
            if mc % 2 == 0:
                nc.scalar.activation(
                    out=g_sb[:, mc, col0:], in_=src,
                    func=mybir.ActivationFunctionType.Copy, scale=1.0,
                )
            else:
                nc.vector.tensor_copy(g_sb[:, mc, col0:], src)

        t1_sb = mid.tile([P, CC, C], FP16)
        tps = {}
        tA = psB.tile([P, 2, C], FP32, tag="psB")
        tps[3], tps[2] = tA[:, 0, :], tA[:, 1, :]
        tB = psB.tile([P, 2, C], FP32, tag="psB")
        tps[1], tps[0] = tB[:, 0, :], tB[:, 1, :]

        def t1_pass(mc):
            # T1 = G @ Wt^T (G blocks stationary). mc=3 uses only triangle
            # rows; other mc need the transposed lower blocks.
            tp = tps[mc]
            for jc in range(CC):
                nc.tensor.matmul(
                    tp,
                    g_sb[:, jc, mc * P:(mc + 1) * P],
                    wthetaT[:, jc, :],
                    start=(jc == 0),
                    stop=(jc == CC - 1),
                )
            # copies are on L's critical path: split across ACT + DVE
            nc.scalar.activation(
                out=t1_sb[:, mc, :C // 2], in_=tp[:, :C // 2],
                func=mybir.ActivationFunctionType.Copy, scale=1.0,
            )
            nc.vector.tensor_copy(t1_sb[:, mc, C // 2:], tp[:, C // 2:])

        t1_pass(3)
        # lower blocks (a > b): G[a, b-block] = T(G[b, a-block]); packed into
        # one fp16 PSUM tile, copied back right after each transpose so each
        # T1 pass unblocks as early as possible (T1(mc) needs blocks (*,mc)).
        # T1 order 3,0,1,2 makes the t1 rows L consumes first finish first.
        gt = psA.tile([P, 2, 4, P], FP16, tag="psA")
        lower = [(1, 0), (2, 0), (3, 0), (2, 1), (3, 1), (3, 2)]

        def g_fill(i):
            a, b = lower[i]
            nc.tensor.transpose(
                gt[:, i // 4, i % 4, :],
                g_sb[:, b, a * P:(a + 1) * P],
                id16,
            )
            nc.any.tensor_copy(
                g_sb[:, a, b * P:(b + 1) * P], gt[:, i // 4, i % 4, :]
            )

        g_fill(0)
        g_fill(1)
        g_fill(2)
        t1_pass(0)
        g_fill(3)
        g_fill(4)
        t1_pass(1)
        g_fill(5)
        t1_pass(2)

        # ---- L = Wp @ T1 ; softmax rows -> attn (fp16); attn^T transposes
        # interleaved per-mc so the PE never waits on a softmax.
        attn_sb = mid.tile([P, CC, C], FP16)
        attnT16 = mid.tile([P, CC, C], FP16)
        ptA = psA.tile([P, 2, C], FP16, tag="psA")  # attnT rows dc=0,1
        ptB = psA.tile([P, 2, C], FP16, tag="psA")  # attnT rows dc=2,3
        pt = {0: ptA[:, 0, :], 1: ptA[:, 1, :], 2: ptB[:, 0, :], 3: ptB[:, 1, :]}
        lps = {}

        neg_max = {}
        sums = {}
        rinv = {}

        def l_pass(mc):
            lp = lps[mc]
            # ic order follows T1 completion order (3,0,1,2) so L's early
            # matmuls use the t1 rows copied earliest
            seq = [3, 0, 1, 2]
            for ic in seq:
                nc.tensor.matmul(
                    lp,
                    wphiT[:, ic, mc * P:(mc + 1) * P],
                    t1_sb[:, ic, :],
                    start=(ic == seq[0]),
                    stop=(ic == seq[-1]),
                )
            # only the row-max on DVE here: all four maxes queue back-to-back
            # (no reciprocal head-of-line blocking in the DVE FIFO)
            neg_max[mc] = vecs.tile([P, 1], FP32, name=f"negmax{mc}")
            nc.vector.tensor_reduce(
                out=neg_max[mc], in_=lp, axis=mybir.AxisListType.X,
                op=mybir.AluOpType.max, negate=True,
            )

        def sm_exp(mc):
            sums[mc] = vecs.tile([P, 1], FP32, name=f"sums{mc}")
            nc.scalar.activation(
                out=attn_sb[:, mc, :], in_=lps[mc],
                func=mybir.ActivationFunctionType.Exp,
                bias=neg_max[mc], scale=1.0, accum_out=sums[mc],
            )

        def sm_recip(mc):
            rinv[mc] = vecs.tile([P, 1], FP32, name=f"rinv{mc}")
            nc.vector.reciprocal(rinv[mc], sums[mc])

        def sm_mul(mc):
            nc.vector.tensor_scalar_mul(
                attn_sb[:, mc, :], attn_sb[:, mc, :], rinv[mc]
            )

        def at_pass(mc):
            for dc in range(CC):
                nc.tensor.transpose(
                    pt[dc][:, mc * P:(mc + 1) * P],
                    attn_sb[:, mc, dc * P:(dc + 1) * P],
                    id16,
                )

        # L order 3,0,1,2: softmax(3) (the attnT straggler otherwise) hides
        # under L(0)/L(1); each at_pass only needs its own softmax done.
        lpA = psB.tile([P, 2, C], FP32, tag="psB")
        lpB = psB.tile([P, 2, C], FP32, tag="psB")
        lps[3], lps[0] = lpA[:, 0, :], lpA[:, 1, :]
        lps[1], lps[2] = lpB[:, 0, :], lpB[:, 1, :]
        # Emission tuned for engine FIFO order: DVE sees max3,max0,max1,
        # recip3,mul3,max2,recip0,mul0,... so every op is ready when it
        # reaches the queue head; aT(3) slots into PE order before l(2).
        l_pass(3)
        sm_exp(3)
        l_pass(0)
        sm_exp(0)
        l_pass(1)
        sm_exp(1)
        sm_recip(3)
        sm_mul(3)
        at_pass(3)
        l_pass(2)
        sm_exp(2)
        sm_recip(0)
        sm_mul(0)
        # p-state filler in the mul(0) wait (writes lps[3]: reads done)
        for _ in range(3):
            nc.tensor.matmul(
                lps[3], wsrc[:, :P], wsrc, start=True, stop=True
            )
        at_pass(0)
        sm_recip(1)
        sm_mul(1)
        at_pass(1)
        sm_recip(2)
        sm_mul(2)
        at_pass(2)
        for dc in range(CC):
            nc.any.tensor_copy(attnT16[:, dc, :], pt[dc])
        # p-state filler: keep the PE clocked through the attnT-copy bubble
        # so the A'-fold matmuls run at full rate. Reuses lps[1]'s PSUM slice
        # (its softmax reads finished long ago) to avoid any WAR stall.
        for _ in range(3):
            nc.tensor.matmul(
                lps[1], wsrc[:, :P], wsrc, start=True, stop=True
            )

        # ---- A'^T[j, c] = sum_d Wg[d, j] attn[c, d]; cast to fp8 * S_A
        apT8 = mid.tile([P, CC, C], FP8)
        for half in range(2):
            ap_ps = psB.tile([P, 2, C], FP32, tag="psB")
            for sub in range(2):
                jc = half * 2 + sub
                for dc in range(CC):
                    nc.tensor.matmul(
                        ap_ps[:, sub, :],
                        wg16[:, dc, jc * P:(jc + 1) * P],
                        attnT16[:, dc, :],
                        start=(dc == 0),
                        stop=(dc == CC - 1),
                    )
                nc.scalar.activation(
                    out=apT8[:, jc, :], in_=ap_ps[:, sub, :],
                    func=mybir.ActivationFunctionType.Copy, scale=S_A,
                )

        # ---- ZS (y^T blocks in Z layout) + mask GEMM + residual + store,
        # both in fp8 DoubleRow (K=256 per pass). Software-pipelined:
        # mask(q-1) runs on PE while ZS(q)'s PSUM->fp8 casts run on ACT/DVE.
        zs8 = mid.tile([P, CC, N], FP8)
        xr8 = x8_sb.rearrange("p cc (ci m q) -> p cc ci q m", ci=CC, q=QF)

        def zs_half(q, ci2):
            # 4 DoubleRow matmuls -> one ACT cast to fp8 (Z * S_X)
            pool = psA if ci2 == 0 else psB
            zp = pool.tile([P, 2, C], FP32, tag="psA" if ci2 == 0 else "psB")
            for s in range(2):
                ci = ci2 * 2 + s
                for j2 in range(2):
                    nc.tensor.matmul(
                        zp[:, s, :],
                        xr8[:, 2 * j2:2 * j2 + 2, ci, q, :],
                        apT8[:, 2 * j2:2 * j2 + 2, :],
                        start=(j2 == 0),
                        stop=(j2 == 1),
                        perf_mode=mybir.MatmulPerfMode.DoubleRow,
                    )
            nc.scalar.activation(
                out=zs8[:, 2 * ci2:2 * ci2 + 2, q * C:(q + 1) * C],
                in_=zp, func=mybir.ActivationFunctionType.Copy,
                scale=ZCAST,
            )

        def mask_half(q, oc2, prefill=False):
            # 4 DoubleRow matmuls (PSUM = mask * S_MX), then one fused DVE
            # op: out = PSUM/S_MX + x (residual), straight to fp16.
            # The last q instead prefills the residual into PSUM (ACT) and
            # finishes with an ACT scale-copy, keeping the tail off the
            # DVE FIFO (its MULTIPLY,ADDs would serialize the epilogue).
            pool = psA if oc2 == 0 else psB
            mp = pool.tile([P, 2, C], FP32, tag="psA" if oc2 == 0 else "psB")
            if prefill:
                nc.scalar.activation(
                    out=mp,
                    in_=x16_sb[:, 2 * oc2:2 * oc2 + 2, q * C:(q + 1) * C],
                    func=mybir.ActivationFunctionType.Copy, scale=S_MX,
                )
            for s in range(2):
                oc = oc2 * 2 + s
                for i2 in range(2):
                    nc.tensor.matmul(
                        mp[:, s, :],
                        wmT8[:, 2 * i2:2 * i2 + 2, oc * P:(oc + 1) * P],
                        zs8[:, 2 * i2:2 * i2 + 2, q * C:(q + 1) * C],
                        start=(i2 == 0 and not prefill),
                        stop=(i2 == 1),
                        perf_mode=mybir.MatmulPerfMode.DoubleRow,
                        skip_group_check=prefill,
                    )
            ot = outp.tile([P, 2, C], FP16)
            if prefill:
                nc.scalar.activation(
                    out=ot, in_=mp,
                    func=mybir.ActivationFunctionType.Copy, scale=1.0 / S_MX,
                )
            else:
                nc.vector.scalar_tensor_tensor(
                    out=ot, in0=mp, scalar=1.0 / S_MX,
                    in1=x16_sb[:, 2 * oc2:2 * oc2 + 2, q * C:(q + 1) * C],
                    op0=mybir.AluOpType.mult, op1=mybir.AluOpType.add,
                )
            blk = (q * 2 + oc2) * 2 * C
            nc.sync.dma_start(out=out_d[:, blk:blk + 2 * C], in_=ot)

        for q in range(QF):
            zs_half(q, 0)
            if q > 0:
                mask_half(q - 1, 0)
            zs_half(q, 1)
            if q > 0:
                mask_half(q - 1, 1)
        mask_half(QF - 1, 0, prefill=True)
        mask_half(QF - 1, 1, prefill=True)


_NC_CACHE = {}
LAST_RESULT = None


def get_nc():
    if "nc" not in _NC_CACHE:
        _NC_CACHE["nc"] = _build_nc()
    return _NC_CACHE["nc"]


def _e4m3(a):
    return np.asarray(
        np.clip(np.asarray(a, np.float32), -448.0, 448.0),
        ml_dtypes.float8_e4m3fn,
    )


_ID16 = np.eye(P, dtype=np.float16)


def _pack(arr2d):
    """[cc*128, W] -> [128, cc*W]: per-partition-contiguous SBUF layout."""
    a = np.asarray(arr2d)
    cc = a.shape[0] // P
    return np.ascontiguousarray(
        a.reshape(cc, P, a.shape[1]).transpose(1, 0, 2).reshape(P, -1)
    )


def make_in_map(xb, w_phi_tp, w_theta_tp, w_gp, w_mask_tp8):
    """Per-core input dict; xb is one sample [C, H, W]."""
    xf = np.ascontiguousarray(xb.reshape(C, N), dtype=np.float32)
    return {
        "id16": _ID16,
        "xt16p": _pack(np.ascontiguousarray(xf.T).astype(np.float16)),
        "x16p": _pack(xf.astype(np.float16)),
        "x8p": _pack(_e4m3(xf * S_X)),
        "w_phi_tp": w_phi_tp,
        "w_theta_tp": w_theta_tp,
        "w_gp": w_gp,
        "w_mask_tp8": w_mask_tp8,
    }


def prep_weights(w_phi, w_theta, w_g, w_mask, gamma):
    w_phi_tp = _pack(np.asarray(w_phi, dtype=np.float32).T.astype(np.float16))
    w_theta_tp = _pack(np.asarray(w_theta, dtype=np.float32).T.astype(np.float16))
    w_gp = _pack(np.asarray(w_g, dtype=np.float32).astype(np.float16))
    gamma64 = float(np.asarray(gamma, dtype=np.float32).reshape(-1)[0])
    w_mask_tp8 = _pack(_e4m3(
        (np.asarray(w_mask, dtype=np.float64).T * gamma64 * S_M).astype(np.float32)
    ))
    return w_phi_tp, w_theta_tp, w_gp, w_mask_tp8


def kernel(x, w_phi, w_theta, w_g, w_mask, gamma):
    global LAST_RESULT
    x = np.ascontiguousarray(np.asarray(x, dtype=np.float32))
    B, c, h, w = x.shape
    assert (c, h * w) == (C, N), (x.shape,)

    w_phi_tp, w_theta_tp, w_gp, w_mask_tp8 = prep_weights(
        w_phi, w_theta, w_g, w_mask, gamma
    )
    nc = get_nc()
    in_maps = [
        make_in_map(x[b], w_phi_tp, w_theta_tp, w_gp, w_mask_tp8)
        for b in range(B)
    ]
    trace = bool(int(os.environ.get("KERNEL_TRACE", "0")))
    res = run_bass_kernel_spmd(nc, in_maps, list(range(B)), trace=trace)
    LAST_RESULT = res
    out = np.empty((B, c, h * w), dtype=np.float32)
    for b in range(B):
        a = np.asarray(res.results[b]["outp"], dtype=np.float32)
        # [p, (q, oc2, s, c)] -> channel = (2*oc2+s)*128+p, col = q*512+c
        out[b] = a.reshape(P, QF, 2, 2, C).transpose(2, 3, 0, 1, 4).reshape(c, h * w)
    return out.reshape(B, c, h, w)


# revision 40
# speedup vs baseline: 1.0327x; 1.0240x over previous
# Trainium2 Bass kernel for per-sample channel-attention module (CAM).
#
# Reference math per sample (C=512, N=H*W=4096):
#   X = x.reshape(C, N)
#   phi = Wp X ; theta = Wt X ; g = Wg X
#   attn = softmax_rows(phi @ theta^T)          # [C, C]
#   y = attn @ g                                 # [C, N]
#   Z = (y^T).flatten().reshape(C, N)            # torch permute+view reinterpretation
#   out = gamma * (Wm @ Z) + x
#
# Algebraic restructuring (cuts PE work ~1.8x vs the naive 6-GEMM chain):
#   G = X X^T                  (Gram, [C, C])
#   L = Wp G Wt^T              (attention logits via two small GEMMs)
#   A' = softmax(L) @ Wg       (fold g-projection into attention)
#   y = A' X                   (single big GEMM)
# The torch permute+view reinterpretation is free: y^T blocks are produced
# with a stride-8 column selection of X as the stationary matmul operand, so
# each PSUM tile lands exactly on a contiguous block of Z's SBUF layout.
#
# Mixed precision (validated against the fp64 reference; the softmax here is
# a hard argmax with large top1-top2 logit gaps, so post-softmax stages are
# linear in quantization error while the logit path needs >=10 bits):
#   - logit path (X^T stream, Gram, G, T1, Wp, Wt): fp16 — fp16 weights get
#     the automatic fast-weight-load path so LDWEIGHTS hides behind matmuls
#   - attn: fp16 (fp32 PSUM + exact max-subtraction in the softmax)
#   - A', X, Z, gamma*Wm^T: fp8 e4m3 with power-of-2 scales; ZS and mask
#     GEMMs run in DoubleRow perf mode (K=256 per pass, ~2x fp16 rate)
#   - residual: fp16 x, fused into the mask PSUM drain as one DVE
#     scalar_tensor_tensor op (out = PSUM/2^16 + x -> fp16); the last q uses
#     an ACT prefill + ACT scale-copy instead to keep the tail off the DVE
# Gram exploits symmetry: only upper-triangle blocks are computed (1280 of
# 2048 moving columns per tile); the 6 lower blocks come from PE transposes.

import os
import numpy as np
import ml_dtypes

import concourse.bass as bass
import concourse.mybir as mybir
import concourse.tile as tile
from concourse import bacc
from concourse.bass_utils import run_bass_kernel_spmd
from concourse.tile import TileContext

P = 128          # partitions
C = 512          # channels
N = 4096         # spatial (64*64)
CC = C // P      # 4 channel chunks
NT = N // P      # 32 spatial tiles
QF = N // C      # 8 fold factor for the permute+view reinterpretation
FP32 = mybir.dt.float32
FP16 = mybir.dt.float16
FP8 = mybir.dt.float8e4

S_X = 8.0        # x fp8 scale
S_A = 512.0      # A' fp8 scale
S_M = 8192.0     # gamma*Wm^T fp8 scale
S_MX = S_M * S_X           # 65536: residual prefill scale
ZCAST = S_X / (S_A * S_X)  # PSUM (y*S_A*S_X) -> Z*S_X


def _build_nc():
    # All DRAM tensors are HOST-PACKED into the exact SBUF layout
    # ([128 partitions, flat free dim]) so every DMA descriptor moves a
    # 2-16KB contiguous run — 1KB fp16 rows were descriptor-rate-bound.
    nc = bacc.Bacc("TRN2", target_bir_lowering=False, debug=False, num_devices=8)
    id_d = nc.dram_tensor("id16", [P, P], FP16, kind="ExternalInput").ap()
    xt_d = nc.dram_tensor("xt16p", [P, NT * C], FP16, kind="ExternalInput").ap()
    x16_d = nc.dram_tensor("x16p", [P, CC * N], FP16, kind="ExternalInput").ap()
    x8_d = nc.dram_tensor("x8p", [P, CC * N], FP8, kind="ExternalInput").ap()
    wphiT_d = nc.dram_tensor("w_phi_tp", [P, CC * C], FP16, kind="ExternalInput").ap()
    wthetaT_d = nc.dram_tensor("w_theta_tp", [P, CC * C], FP16, kind="ExternalInput").ap()
    wg_d = nc.dram_tensor("w_gp", [P, CC * C], FP16, kind="ExternalInput").ap()
    wmT8_d = nc.dram_tensor("w_mask_tp8", [P, CC * C], FP8, kind="ExternalInput").ap()
    # out rows: (q, oc2, s, c) per partition; host unpacks to [C, N]
    out_d = nc.dram_tensor("outp", [P, QF * 2 * 2 * C], FP16, kind="ExternalOutput").ap()

    with TileContext(nc) as tc:
        _body(tc, id_d, xt_d, x16_d, x8_d, wphiT_d, wthetaT_d, wg_d, wmT8_d, out_d)
    nc.compile()
    return nc


def _body(tc, id_d, xt_d, x16_d, x8_d, wphiT_d, wthetaT_d, wg_d, wmT8_d, out_d):
    nc = tc.nc
    from contextlib import ExitStack

    with ExitStack() as ctx:
        const = ctx.enter_context(tc.tile_pool(name="const", bufs=1))
        xtp = ctx.enter_context(tc.tile_pool(name="xtp", bufs=1))
        xin = ctx.enter_context(tc.tile_pool(name="xin", bufs=1))
        wpool = ctx.enter_context(tc.tile_pool(name="wpool", bufs=1))
        mid = ctx.enter_context(tc.tile_pool(name="mid", bufs=1))
        vecs = ctx.enter_context(tc.tile_pool(name="vecs", bufs=8))
        outp = ctx.enter_context(tc.tile_pool(name="outp", bufs=4))
        psA = ctx.enter_context(tc.tile_pool(name="psA", bufs=2, space="PSUM"))
        psB = ctx.enter_context(tc.tile_pool(name="psB", bufs=2, space="PSUM"))

        # Warm source: DVE memset (no gpsimd in the startup path). ~18
        # throwaway matmuls ramp the PE p-state while DMA streams in.
        wsrc = const.tile([P, C], FP16)
        nc.vector.memset(wsrc, 1.0)
        warm = psB.tile([P, 2, C], FP32, tag="psB")
        for _ in range(11):
            nc.tensor.matmul(
                warm[:, 0, :], wsrc[:, :P], wsrc, start=True, stop=True
            )

        # ---- input DMA schedule.
        # sync HWDGE:   xt even chunks, x16 left half; output stores later.
        # scalar HWDGE: identity, xt odd chunks, w_theta, w_phi, wg, x16 right.
        # gpsimd SWDGE: x8, wmT8 (needed latest; software queue).
        id16 = const.tile([P, P], FP16)
        xt_sb = xtp.tile([P, NT, C], FP16)
        x16_sb = xin.tile([P, CC, N], FP16)
        x8_sb = xin.tile([P, CC, N], FP8)
        wphiT = wpool.tile([P, CC, C], FP16)
        wthetaT = wpool.tile([P, CC, C], FP16)
        wg16 = wpool.tile([P, CC, C], FP16)
        wmT8 = wpool.tile([P, CC, C], FP8)

        # xt in 8 four-tile chunks (4KB/partition each) alternating the two
        # HWDGE queues; weights/identity/x8 on the SWDGE path.
        NCHUNK = 8
        TPC = NT // NCHUNK
        CW = TPC * C
        for k in range(NCHUNK):
            eng = nc.sync if (k % 2 == 0) else nc.scalar
            eng.dma_start(
                out=xt_sb[:, k * TPC:(k + 1) * TPC, :],
                in_=xt_d[:, k * CW:(k + 1) * CW],
            )
        nc.gpsimd.dma_start(out=id16, in_=id_d)
        nc.gpsimd.dma_start(out=wthetaT, in_=wthetaT_d)
        nc.gpsimd.dma_start(out=wphiT, in_=wphiT_d)
        nc.gpsimd.dma_start(out=wg16, in_=wg_d)
        nc.gpsimd.dma_start(out=wmT8, in_=wmT8_d)
        nc.gpsimd.dma_start(out=x8_sb, in_=x8_d)
        HN = N // 2
        xv = x16_d.rearrange("p (cc n) -> p cc n", cc=CC)
        nc.sync.dma_start(out=x16_sb[:, :, :HN], in_=xv[:, :, :HN])
        nc.scalar.dma_start(out=x16_sb[:, :, HN:], in_=xv[:, :, HN:])

        # ---- Gram, upper triangle only: G[mc-block, 128*mc:] accumulated
        # over the 32 streamed xt tiles. gaccA = rows 0,1; gaccB = rows 2,3.
        gaccA = psA.tile([P, 2, C], FP32, tag="psA")
        gaccB = psA.tile([P, 2, C], FP32, tag="psA")
        gacc = [
            (gaccA[:, 0, :], 0), (gaccA[:, 1, :C - P], P),
            (gaccB[:, 0, :C - 2 * P], 2 * P), (gaccB[:, 1, :C - 3 * P], 3 * P),
        ]
        for t in range(NT):
            for mc in range(CC):
                dst, col0 = gacc[mc]
                nc.tensor.matmul(
                    dst,
                    xt_sb[:, t, mc * P:(mc + 1) * P],
                    xt_sb[:, t, col0:],
                    start=(t == 0),
                    stop=(t == NT - 1),
                )

        # full G (fp16) in SBUF: triangle rows + 6 transposed lower blocks
        g_sb = mid.tile([P, CC, C], FP16)
        for mc in range(CC):
            src, col0 = gacc[mc]
            if mc % 2 == 0:
                nc.scalar.activation(
                    out=g_sb[:, mc, col0:], in_=src,
                    func=mybir.ActivationFunctionType.Copy, scale=1.0,
                )
            else:
                nc.vector.tensor_copy(g_sb[:, mc, col0:], src)

        t1_sb = mid.tile([P, CC, C], FP16)
        tps = {}
        tA = psB.tile([P, 2, C], FP32, tag="psB")
        tps[3], tps[2] = tA[:, 0, :], tA[:, 1, :]
        tB = psB.tile([P, 2, C], FP32, tag="psB")
        tps[1], tps[0] = tB[:, 0, :], tB[:, 1, :]

        def t1_pass(mc):
            # T1 = G @ Wt^T (G blocks stationary). mc=3 uses only triangle
            # rows; other mc need the transposed lower blocks.
            tp = tps[mc]
            for jc in range(CC):
                nc.tensor.matmul(
                    tp,
                    g_sb[:, jc, mc * P:(mc + 1) * P],
                    wthetaT[:, jc, :],
                    start=(jc == 0),
                    stop=(jc == CC - 1),
                )
            # copies are on L's critical path: split across ACT + DVE
            nc.scalar.activation(
                out=t1_sb[:, mc, :C // 2], in_=tp[:, :C // 2],
                func=mybir.ActivationFunctionType.Copy, scale=1.0,
            )
            nc.vector.tensor_copy(t1_sb[:, mc, C // 2:], tp[:, C // 2:])

        # lower blocks (a > b): G[a, b-block] = T(G[b, a-block]); packed into
        # one fp16 PSUM tile. The (x,0) transposes read only G row 0 (the
        # first row copied), so they run BEFORE T1(3) and their copies drain
        # on ACT/DVE during T1(3)'s matmuls — T1(0) starts unblocked.
        # T1 order 3,0,1,2 makes the t1 rows L consumes first finish first.
        gt = psA.tile([P, 2, 4, P], FP16, tag="psA")
        lower = [(1, 0), (2, 0), (3, 0), (2, 1), (3, 1), (3, 2)]

        def g_fill(i):
            a, b = lower[i]
            nc.tensor.transpose(
                gt[:, i // 4, i % 4, :],
                g_sb[:, b, a * P:(a + 1) * P],
                id16,
            )
            nc.any.tensor_copy(
                g_sb[:, a, b * P:(b + 1) * P], gt[:, i // 4, i % 4, :]
            )

        g_fill(0)
        g_fill(1)
        g_fill(2)
        t1_pass(3)
        g_fill(3)
        g_fill(4)
        t1_pass(0)
        g_fill(5)
        t1_pass(1)
        t1_pass(2)

        # ---- L = Wp @ T1 ; softmax rows -> attn (fp16); attn^T transposes
        # interleaved per-mc so the PE never waits on a softmax.
        attn_sb = mid.tile([P, CC, C], FP16)
        attnT16 = mid.tile([P, CC, C], FP16)
        ptA = psA.tile([P, 2, C], FP16, tag="psA")  # attnT rows dc=0,1
        ptB = psA.tile([P, 2, C], FP16, tag="psA")  # attnT rows dc=2,3
        pt = {0: ptA[:, 0, :], 1: ptA[:, 1, :], 2: ptB[:, 0, :], 3: ptB[:, 1, :]}
        lps = {}

        neg_max = {}
        sums = {}
        rinv = {}

        def l_pass(mc):
            lp = lps[mc]
            # ic order follows T1 completion order (3,0,1,2) so L's early
            # matmuls use the t1 rows copied earliest
            seq = [3, 0, 1, 2]
            for ic in seq:
                nc.tensor.matmul(
                    lp,
                    wphiT[:, ic, mc * P:(mc + 1) * P],
                    t1_sb[:, ic, :],
                    start=(ic == seq[0]),
                    stop=(ic == seq[-1]),
                )
            # only the row-max on DVE here: all four maxes queue back-to-back
            # (no reciprocal head-of-line blocking in the DVE FIFO)
            neg_max[mc] = vecs.tile([P, 1], FP32, name=f"negmax{mc}")
            nc.vector.tensor_reduce(
                out=neg_max[mc], in_=lp, axis=mybir.AxisListType.X,
                op=mybir.AluOpType.max, negate=True,
            )

        def sm_exp(mc):
            sums[mc] = vecs.tile([P, 1], FP32, name=f"sums{mc}")
            nc.scalar.activation(
                out=attn_sb[:, mc, :], in_=lps[mc],
                func=mybir.ActivationFunctionType.Exp,
                bias=neg_max[mc], scale=1.0, accum_out=sums[mc],
            )

        def sm_recip(mc):
            rinv[mc] = vecs.tile([P, 1], FP32, name=f"rinv{mc}")
            nc.vector.reciprocal(rinv[mc], sums[mc])

        def sm_mul(mc):
            nc.vector.tensor_scalar_mul(
                attn_sb[:, mc, :], attn_sb[:, mc, :], rinv[mc]
            )

        def at_pass(mc):
            for dc in range(CC):
                nc.tensor.transpose(
                    pt[dc][:, mc * P:(mc + 1) * P],
                    attn_sb[:, mc, dc * P:(dc + 1) * P],
                    id16,
                )

        # L order 3,0,1,2: softmax(3) (the attnT straggler otherwise) hides
        # under L(0)/L(1); each at_pass only needs its own softmax done.
        lpA = psB.tile([P, 2, C], FP32, tag="psB")
        lpB = psB.tile([P, 2, C], FP32, tag="psB")
        lps[3], lps[0] = lpA[:, 0, :], lpA[:, 1, :]
        lps[1], lps[2] = lpB[:, 0, :], lpB[:, 1, :]
        # Emission tuned for engine FIFO order: DVE sees max3,max0,max1,
        # recip3,mul3,max2,recip0,mul0,... so every op is ready when it
        # reaches the queue head; aT(3) slots into PE order before l(2).
        l_pass(3)
        sm_exp(3)
        l_pass(0)
        sm_exp(0)
        l_pass(1)
        sm_exp(1)
        sm_recip(3)
        sm_mul(3)
        at_pass(3)
        l_pass(2)
        sm_exp(2)
        sm_recip(0)
        sm_mul(0)
        # p-state filler in the mul(0) wait (writes lps[3]: reads done)
        for _ in range(3):
            nc.tensor.matmul(
                lps[3], wsrc[:, :P], wsrc, start=True, stop=True
            )
        at_pass(0)
        sm_recip(1)
        sm_mul(1)
        at_pass(1)
        sm_recip(2)
        sm_mul(2)
        at_pass(2)
        for dc in range(CC):
            nc.any.tensor_copy(attnT16[:, dc, :], pt[dc])
        # p-state filler: keep the PE clocked through the attnT-copy bubble
        # so the A'-fold matmuls run at full rate. Reuses lps[1]'s PSUM slice
        # (its softmax reads finished long ago) to avoid any WAR stall.
        for _ in range(3):
            nc.tensor.matmul(
                lps[1], wsrc[:, :P], wsrc, start=True, stop=True
            )

        # ---- A'^T[j, c] = sum_d Wg[d, j] attn[c, d]; cast to fp8 * S_A
        apT8 = mid.tile([P, CC, C], FP8)
        for half in range(2):
            ap_ps = psB.tile([P, 2, C], FP32, tag="psB")
            for sub in range(2):
                jc = half * 2 + sub
                for dc in range(CC):
                    nc.tensor.matmul(
                        ap_ps[:, sub, :],
                        wg16[:, dc, jc * P:(jc + 1) * P],
                        attnT16[:, dc, :],
                        start=(dc == 0),
                        stop=(dc == CC - 1),
                    )
                nc.scalar.activation(
                    out=apT8[:, jc, :], in_=ap_ps[:, sub, :],
                    func=mybir.ActivationFunctionType.Copy, scale=S_A,
                )

        # ---- ZS (y^T blocks in Z layout) + mask GEMM + residual + store,
        # both in fp8 DoubleRow (K=256 per pass). Software-pipelined:
        # mask(q-1) runs on PE while ZS(q)'s PSUM->fp8 casts run on ACT/DVE.
        zs8 = mid.tile([P, CC, N], FP8)
        xr8 = x8_sb.rearrange("p cc (ci m q) -> p cc ci q m", ci=CC, q=QF)

        def zs_half(q, ci2):
            # 4 DoubleRow matmuls -> one ACT cast to fp8 (Z * S_X)
            pool = psA if ci2 == 0 else psB
            zp = pool.tile([P, 2, C], FP32, tag="psA" if ci2 == 0 else "psB")
            for s in range(2):
                ci = ci2 * 2 + s
                for j2 in range(2):
                    nc.tensor.matmul(
                        zp[:, s, :],
                        xr8[:, 2 * j2:2 * j2 + 2, ci, q, :],
                        apT8[:, 2 * j2:2 * j2 + 2, :],
                        start=(j2 == 0),
                        stop=(j2 == 1),
                        perf_mode=mybir.MatmulPerfMode.DoubleRow,
                    )
            nc.scalar.activation(
                out=zs8[:, 2 * ci2:2 * ci2 + 2, q * C:(q + 1) * C],
                in_=zp, func=mybir.ActivationFunctionType.Copy,
                scale=ZCAST,
            )

        def mask_half(q, oc2, prefill=False):
            # 4 DoubleRow matmuls (PSUM = mask * S_MX), then one fused DVE
            # op: out = PSUM/S_MX + x (residual), straight to fp16.
            # The last q instead prefills the residual into PSUM (ACT) and
            # finishes with an ACT scale-copy, keeping the tail off the
            # DVE FIFO (its MULTIPLY,ADDs would serialize the epilogue).
            pool = psA if oc2 == 0 else psB
            mp = pool.tile([P, 2, C], FP32, tag="psA" if oc2 == 0 else "psB")
            if prefill:
                nc.scalar.activation(
                    out=mp,
                    in_=x16_sb[:, 2 * oc2:2 * oc2 + 2, q * C:(q + 1) * C],
                    func=mybir.ActivationFunctionType.Copy, scale=S_MX,
                )
            for s in range(2):
                oc = oc2 * 2 + s
                for i2 in range(2):
                    nc.tensor.matmul(
                        mp[:, s, :],
                        wmT8[:, 2 * i2:2 * i2 + 2, oc * P:(oc + 1) * P],
                        zs8[:, 2 * i2:2 * i2 + 2, q * C:(q + 1) * C],
                        start=(i2 == 0 and not prefill),
                        stop=(i2 == 1),
                        perf_mode=mybir.MatmulPerfMode.DoubleRow,
                        skip_group_check=prefill,
                    )
            ot = outp.tile([P, 2, C], FP16)
            if prefill:
                nc.scalar.activation(
                    out=ot, in_=mp,
                    func=mybir.ActivationFunctionType.Copy, scale=1.0 / S_MX,
                )
            else:
                nc.vector.scalar_tensor_tensor(
                    out=ot, in0=mp, scalar=1.0 / S_MX,
                    in1=x16_sb[:, 2 * oc2:2 * oc2 + 2, q * C:(q + 1) * C],
                    op0=mybir.AluOpType.mult, op1=mybir.AluOpType.add,
                )
            blk = (q * 2 + oc2) * 2 * C
            nc.sync.dma_start(out=out_d[:, blk:blk + 2 * C], in_=ot)

        for q in range(QF):
            zs_half(q, 0)
            if q > 0:
                mask_half(q - 1, 0)
            zs_half(q, 1)
            if q > 0:
                mask_half(q - 1, 1)
        mask_half(QF - 1, 0, prefill=True)
        mask_half(QF - 1, 1, prefill=True)


_NC_CACHE = {}
LAST_RESULT = None


def get_nc():
    if "nc" not in _NC_CACHE:
        _NC_CACHE["nc"] = _build_nc()
    return _NC_CACHE["nc"]


def _e4m3(a):
    return np.asarray(
        np.clip(np.asarray(a, np.float32), -448.0, 448.0),
        ml_dtypes.float8_e4m3fn,
    )


_ID16 = np.eye(P, dtype=np.float16)


def _pack(arr2d):
    """[cc*128, W] -> [128, cc*W]: per-partition-contiguous SBUF layout."""
    a = np.asarray(arr2d)
    cc = a.shape[0] // P
    return np.ascontiguousarray(
        a.reshape(cc, P, a.shape[1]).transpose(1, 0, 2).reshape(P, -1)
    )


def make_in_map(xb, w_phi_tp, w_theta_tp, w_gp, w_mask_tp8):
    """Per-core input dict; xb is one sample [C, H, W]."""
    xf = np.ascontiguousarray(xb.reshape(C, N), dtype=np.float32)
    return {
        "id16": _ID16,
        "xt16p": _pack(np.ascontiguousarray(xf.T).astype(np.float16)),
        "x16p": _pack(xf.astype(np.float16)),
        "x8p": _pack(_e4m3(xf * S_X)),
        "w_phi_tp": w_phi_tp,
        "w_theta_tp": w_theta_tp,
        "w_gp": w_gp,
        "w_mask_tp8": w_mask_tp8,
    }


def prep_weights(w_phi, w_theta, w_g, w_mask, gamma):
    w_phi_tp = _pack(np.asarray(w_phi, dtype=np.float32).T.astype(np.float16))
    w_theta_tp = _pack(np.asarray(w_theta, dtype=np.float32).T.astype(np.float16))
    w_gp = _pack(np.asarray(w_g, dtype=np.float32).astype(np.float16))
    gamma64 = float(np.asarray(gamma, dtype=np.float32).reshape(-1)[0])
    w_mask_tp8 = _pack(_e4m3(
        (np.asarray(w_mask, dtype=np.float64).T * gamma64 * S_M).astype(np.float32)
    ))
    return w_phi_tp, w_theta_tp, w_gp, w_mask_tp8


def kernel(x, w_phi, w_theta, w_g, w_mask, gamma):
    global LAST_RESULT
    x = np.ascontiguousarray(np.asarray(x, dtype=np.float32))
    B, c, h, w = x.shape
    assert (c, h * w) == (C, N), (x.shape,)

    w_phi_tp, w_theta_tp, w_gp, w_mask_tp8 = prep_weights(
        w_phi, w_theta, w_g, w_mask, gamma
    )
    nc = get_nc()
    in_maps = [
        make_in_map(x[b], w_phi_tp, w_theta_tp, w_gp, w_mask_tp8)
        for b in range(B)
    ]
    trace = bool(int(os.environ.get("KERNEL_TRACE", "0")))
    res = run_bass_kernel_spmd(nc, in_maps, list(range(B)), trace=trace)
    LAST_RESULT = res
    out = np.empty((B, c, h * w), dtype=np.float32)
    for b in range(B):
        a = np.asarray(res.results[b]["outp"], dtype=np.float32)
        # [p, (q, oc2, s, c)] -> channel = (2*oc2+s)*128+p, col = q*512+c
        out[b] = a.reshape(P, QF, 2, 2, C).transpose(2, 3, 0, 1, 4).reshape(c, h * w)
    return out.reshape(B, c, h, w)
